# revision 1
# baseline (speedup 1.0000x reference)
import numpy as np, sys, os, math, functools
sys.path.insert(0, "/opt/trn_rl_repo")

V, D, L = 32000, 1024, 8
HQ, HKV, HD = 16, 4, 64
H = 2752
B, S = 2, 1024
WINDOW, GEVERY = 256, 4
EPS, BASE = 1e-6, 10000.0
NCORES = 8
T = 256
VSH = V // NCORES
NEG = -30000.0
SCALE = 1.0 / 8.0
HCHUNKS = [(i * 128, 128) for i in range(21)] + [(2688, 64)]

def _core_blocks(c):
    cp = c % 4
    return cp, 7 - cp

def _inv_freq():
    return 1.0 / (BASE ** (np.arange(0, HD, 2, dtype=np.float64) / HD))

def _host_masks(c):
    bA, bB = _core_blocks(c)
    p = np.arange(128)
    qA = bA * 128 + p
    qB = bB * 128 + p
    def mk(qpos, k0, w, local):
        k = k0 + np.arange(w)
        valid = k[None, :] <= qpos[:, None]
        if local:
            valid &= (qpos[:, None] - k[None, :]) < WINDOW
        return np.where(valid, 0.0, NEG).astype(np.float16)
    return (mk(qA, 0, 512, False), mk(qB, 512, 512, False),
            mk(qA, 0, 512, True), mk(qB, 256, 768, True))

def _host_prep(idx, emb, Wq, Wk, Wv, Wo, w1, w3, w2, n1, n2, nf):
    idx = np.asarray(idx); emb = np.asarray(emb, dtype=np.float32)
    invf = _inv_freq()
    in_maps = []
    n1h = np.ascontiguousarray(np.asarray(n1).reshape(L, 8, 128).transpose(0, 2, 1))
    n2h = np.ascontiguousarray(np.asarray(n2).reshape(L, 8, 128).transpose(0, 2, 1))
    nfh = np.ascontiguousarray(np.asarray(nf).reshape(8, 128).T)
    for c in range(NCORES):
        s = c // 4
        bA, bB = _core_blocks(c)
        tok = np.concatenate([idx[s, bA*128:(bA+1)*128], idx[s, bB*128:(bB+1)*128]])
        x0T = np.ascontiguousarray(emb[tok].T)
        pos = np.concatenate([bA*128 + np.arange(128), bB*128 + np.arange(128)])
        ang = pos[:, None].astype(np.float64) * invf[None, :]
        cosq = np.cos(ang).astype(np.float32).reshape(2, 128, 32)
        sinq = np.sin(ang).astype(np.float32).reshape(2, 128, 32)
        mAg, mBg, mAl, mBl = _host_masks(c)
        embT = np.ascontiguousarray(emb[c*VSH:(c+1)*VSH].T.astype(np.float16))
        in_maps.append({
            "x0T": x0T,
            "wq": np.asarray(Wq, dtype=np.float32), "wk": np.asarray(Wk, dtype=np.float32),
            "wv": np.asarray(Wv, dtype=np.float32), "wo": np.asarray(Wo, dtype=np.float32),
            "w1": np.asarray(w1, dtype=np.float32), "w3": np.asarray(w3, dtype=np.float32),
            "w2": np.asarray(w2, dtype=np.float16),
            "n1h": n1h, "n2h": n2h, "nfh": nfh,
            "cosq": cosq, "sinq": sinq,
            "mAg": mAg, "mBg": mBg, "mAl": mAl, "mBl": mBl,
            "embT": embT,
        })
    return in_maps

def _unperm_rows():
    perm = np.zeros(2048, dtype=np.int64)
    for r in range(2048):
        rr, rem = divmod(r, 256)
        slot, p = divmod(rem, 128)
        samp = rr // 4
        bA, bB = _core_blocks(rr)
        blk = bA if slot == 0 else bB
        perm[r] = samp * S + blk * 128 + p
    inv = np.zeros(2048, dtype=np.int64)
    inv[perm] = np.arange(2048)
    return inv

def _assemble(outs):
    full = np.empty((2048, V), dtype=np.float32)
    for c in range(NCORES):
        full[:, c*VSH:(c+1)*VSH] = outs[c]["logits"]
    inv = _unperm_rows()
    return full[inv].reshape(B, S, V)

def _build_nc():
    import concourse.bass as bass
    import concourse.bacc as bacc
    import concourse.mybir as mybir
    from concourse.tile import TileContext
    from concourse.masks import make_identity
    F32, F16, F32R = mybir.dt.float32, mybir.dt.float16, mybir.dt.float32r
    AF = mybir.ActivationFunctionType
    ALU = mybir.AluOpType

    nc = bacc.Bacc("TRN2", target_bir_lowering=False, debug=False, num_devices=NCORES)
    P = {}
    def inp(name, shape, dt=F32):
        P[name] = nc.declare_dram_parameter(name, list(shape), dt, isOutput=False)
    inp("x0T", (D, T))
    inp("wq", (L, D, D)); inp("wk", (L, D, 256)); inp("wv", (L, D, 256)); inp("wo", (L, D, D))
    inp("w1", (L, D, H)); inp("w3", (L, D, H)); inp("w2", (L, H, D), F16)
    inp("n1h", (L, 128, 8)); inp("n2h", (L, 128, 8)); inp("nfh", (128, 8))
    inp("cosq", (2, 128, 32)); inp("sinq", (2, 128, 32))
    inp("mAg", (128, 512), F16); inp("mBg", (128, 512), F16)
    inp("mAl", (128, 512), F16); inp("mBl", (128, 768), F16)
    inp("embT", (D, VSH), F16)
    logits = nc.declare_dram_parameter("logits", [2048, VSH], F32, isOutput=True)

    kv_in  = [nc.dram_tensor(f"kv_in{l}",  [4, 128, 256], F16) for l in range(L)]
    kv_out = [nc.dram_tensor(f"kv_out{l}", [16, 128, 256], F16) for l in range(L)]
    xf_in  = nc.dram_tensor("xf_in", [D, T], F16)
    xf_out = nc.dram_tensor("xf_out", [NCORES * D, T], F16, addr_space="Shared")
    RG_KV = [[0, 1, 2, 3], [4, 5, 6, 7]]
    RG_ALL = [list(range(NCORES))]

    with TileContext(nc) as tc:
      with tc.tile_pool(name="pers", bufs=1) as pers, \
           tc.tile_pool(name="wpool", bufs=2) as wp, \
           tc.tile_pool(name="act", bufs=2) as act, \
           tc.tile_pool(name="attn", bufs=2) as atp, \
           tc.tile_pool(name="small", bufs=4) as sm, \
           tc.tile_pool(name="ppbig", bufs=2, space="PSUM") as ppb, \
           tc.tile_pool(name="ppmid", bufs=2, space="PSUM") as ppm, \
           tc.tile_pool(name="ppo", bufs=2, space="PSUM") as ppo, \
           tc.tile_pool(name="pptr", bufs=2, space="PSUM") as ppt:

        dma = nc.sync.dma_start
        xT = [pers.tile([128, T], F32, tag=f"xT{d}", name=f"xT{d}") for d in range(8)]
        for d in range(8):
            dma(out=xT[d], in_=P["x0T"][d*128:(d+1)*128, :])
        cosA = pers.tile([128, 32], F32, tag="cosA", name="cosA"); dma(out=cosA, in_=P["cosq"][0])
        cosB = pers.tile([128, 32], F32, tag="cosB", name="cosB"); dma(out=cosB, in_=P["cosq"][1])
        sinA = pers.tile([128, 32], F32, tag="sinA", name="sinA"); dma(out=sinA, in_=P["sinq"][0])
        sinB = pers.tile([128, 32], F32, tag="sinB", name="sinB"); dma(out=sinB, in_=P["sinq"][1])
        mAg = pers.tile([128, 512], F16, tag="mAg", name="mAg"); dma(out=mAg, in_=P["mAg"][:, :])
        mBg = pers.tile([128, 512], F16, tag="mBg", name="mBg"); dma(out=mBg, in_=P["mBg"][:, :])
        mAl = pers.tile([128, 512], F16, tag="mAl", name="mAl"); dma(out=mAl, in_=P["mAl"][:, :])
        mBl = pers.tile([128, 768], F16, tag="mBl", name="mBl"); dma(out=mBl, in_=P["mBl"][:, :])
        idn = pers.tile([128, 128], F16, tag="idn", name="idn")
        make_identity(nc, idn)
        onesf = pers.tile([128, 1], F32, tag="onesf", name="onesf")
        nc.vector.memset(onesf, 1.0)
        ones = pers.tile([128, 1], F32R, tag="ones", name="ones")
        nc.vector.tensor_copy(out=ones, in_=onesf)
        epst = pers.tile([1, 1], F32, tag="epst", name="epst")
        nc.vector.memset(epst, EPS)

        def rmsnorm(nw_dram, outdt=F32R):
            nw = sm.tile([128, 8], F32, tag="nw", name="nw")
            dma(out=nw, in_=nw_dram)
            ss = ppm.tile([1, T], F32, tag="pm", name="ss")
            for d in range(8):
                x2 = act.tile([128, T], F32R, tag="x2", name="x2")
                nc.vector.tensor_mul(out=x2, in0=xT[d], in1=xT[d])
                nc.tensor.matmul(ss, lhsT=ones, rhs=x2, start=(d == 0), stop=(d == 7))
            rrow = sm.tile([1, T], F32, tag="rrow", name="rrow")
            nc.scalar.activation(out=rrow, in_=ss, func=AF.Sqrt, scale=1.0/D, bias=epst[0:1, 0:1])
            rrec = sm.tile([1, T], F32, tag="rrec", name="rrec")
            nc.vector.reciprocal(out=rrec, in_=rrow)
            rb = act.tile([128, T], F32, tag="rb", name="rb")
            nc.gpsimd.partition_broadcast(rb[:], rrec[:])
            out = []
            for d in range(8):
                h = act.tile([128, T], outdt, tag=f"hT{d}", name=f"hT{d}", bufs=1)
                nc.vector.scalar_tensor_tensor(out=h, in0=xT[d], scalar=nw[:, d:d+1],
                                               in1=rb, op0=ALU.mult, op1=ALU.mult)
                out.append(h)
            return out

        def rope_tok(ps, cost, sint, outt, nheads):
            ev = ps.rearrange("p (h f two) -> p h f two", two=2, f=32)
            ov = outt.rearrange("p (h f two) -> p h f two", two=2, f=32)
            cb = cost[:].rearrange("p (o f) -> p o f", o=1).to_broadcast((128, nheads, 32))
            sb = sint[:].rearrange("p (o f) -> p o f", o=1).to_broadcast((128, nheads, 32))
            t1 = sm.tile([128, nheads, 32], F32, tag="ropet1", name="ropet1")
            t2 = sm.tile([128, nheads, 32], F32, tag="ropet2", name="ropet2")
            nc.vector.tensor_mul(out=t1, in0=ev[:, :, :, 0], in1=cb)
            nc.vector.tensor_mul(out=t2, in0=ev[:, :, :, 1], in1=sb)
            nc.vector.tensor_sub(out=ov[:, :, :, 0], in0=t1, in1=t2)
            nc.vector.tensor_mul(out=t1, in0=ev[:, :, :, 0], in1=sb)
            nc.vector.tensor_mul(out=t2, in0=ev[:, :, :, 1], in1=cb)
            nc.vector.tensor_add(out=ov[:, :, :, 1], in0=t1, in1=t2)

        for l in range(L):
            is_global = ((l + 1) % GEVERY) == 0
            h1 = rmsnorm(P["n1h"][l])
            # ---- K, V ----
            wkt = wp.tile([128, 8, 256], F32R, tag="wkt", name="wkt", bufs=1)
            wvt = wp.tile([128, 8, 256], F32R, tag="wvt", name="wvt", bufs=1)
            dma(out=wkt, in_=P["wk"][l].rearrange("(dc p) f -> p dc f", p=128).bitcast(F32R))
            dma(out=wvt, in_=P["wv"][l].rearrange("(dc p) f -> p dc f", p=128).bitcast(F32R))
            ktok, vtok = [], []
            for t2_ in range(2):
                psk = ppm.tile([128, 256], F32, tag="pm", name="psk")
                for d in range(8):
                    nc.tensor.matmul(psk, lhsT=h1[d][:, t2_*128:(t2_+1)*128], rhs=wkt[:, d, :],
                                     start=(d == 0), stop=(d == 7))
                kt = atp.tile([128, 256], F16, tag=f"ktok{t2_}", name=f"ktok{t2_}")
                rope_tok(psk, (cosA, cosB)[t2_], (sinA, sinB)[t2_], kt, 4)
                ktok.append(kt)
                psv = ppm.tile([128, 256], F32, tag="pm", name="psv")
                for d in range(8):
                    nc.tensor.matmul(psv, lhsT=h1[d][:, t2_*128:(t2_+1)*128], rhs=wvt[:, d, :],
                                     start=(d == 0), stop=(d == 7))
                vt = atp.tile([128, 256], F16, tag=f"vtok{t2_}", name=f"vtok{t2_}")
                nc.vector.tensor_copy(out=vt, in_=psv)
                vtok.append(vt)
            kT_sb = [atp.tile([128, 256], F16, tag=f"kTsb{i}", name=f"kTsb{i}") for i in range(2)]
            for i in range(2):
                for t2_ in range(2):
                    pst = ppt.tile([128, 128], F16, tag="pstr", name="pstr")
                    nc.tensor.transpose(pst, ktok[t2_][:, i*128:(i+1)*128], idn)
                    nc.vector.tensor_copy(out=kT_sb[i][:, t2_*128:(t2_+1)*128], in_=pst)
            for i in range(2):
                dma(out=kv_in[l][i], in_=kT_sb[i])
                dma(out=kv_in[l][2 + i], in_=vtok[i])
            nc.gpsimd.collective_compute(
                "AllGather", mybir.AluOpType.bypass, replica_groups=RG_KV,
                ins=[kv_in[l].ap()], outs=[kv_out[l].ap()])
            # ---- Q ----
            qtoks = [act.tile([128, D], F16, tag=f"qtok{t}", name=f"qtok{t}", bufs=1) for t in range(2)]
            for hf in range(2):
                wqh = wp.tile([128, 8, 512], F32R, tag="wqh", name="wqh", bufs=1)
                dma(out=wqh, in_=P["wq"][l, :, hf*512:(hf+1)*512].rearrange("(dc p) f -> p dc f", p=128).bitcast(F32R))
                for t2_ in range(2):
                    psq = ppb.tile([128, 512], F32, tag="pb", name="psq")
                    for d in range(8):
                        nc.tensor.matmul(psq, lhsT=h1[d][:, t2_*128:(t2_+1)*128],
                                         rhs=wqh[:, d, :], start=(d == 0), stop=(d == 7))
                    rope_tok(psq, (cosA, cosB)[t2_], (sinA, sinB)[t2_],
                             qtoks[t2_][:, hf*512:(hf+1)*512], 8)
            qT = [atp.tile([128, 256], F16, tag=f"qT{f}", name=f"qT{f}", bufs=1) for f in range(8)]
            for t2_ in range(2):
                for f in range(8):
                    pst = ppt.tile([128, 128], F16, tag="pstr", name="pstr")
                    nc.tensor.transpose(pst, qtoks[t2_][:, f*128:(f+1)*128], idn)
                    if f % 2 == 0:
                        nc.vector.tensor_copy(out=qT[f][:, t2_*128:(t2_+1)*128], in_=pst)
                    else:
                        nc.scalar.activation(out=qT[f][:, t2_*128:(t2_+1)*128], in_=pst, func=AF.Copy)
            # ---- assemble ----
            kT_full = [atp.tile([128, 1024], F16, tag=f"kTf{i}", name=f"kTf{i}", bufs=1) for i in range(2)]
            for i in range(2):
                for b in range(8):
                    r, sl = (b, 0) if b < 4 else (7 - b, 1)
                    dma(out=kT_full[i][:, b*128:(b+1)*128],
                        in_=kv_out[l][r*4 + i, :, sl*128:(sl+1)*128])
            kT_sw = [atp.tile([128, 1024], F16, tag=f"kTw{i}", name=f"kTw{i}", bufs=1) for i in range(2)]
            for i in range(2):
                dma(out=kT_sw[i][0:64, :], in_=kT_full[i][64:128, :])
                dma(out=kT_sw[i][64:128, :], in_=kT_full[i][0:64, :])
            v_full = atp.tile([128, 8, 4, 64], F16, tag="vfull", name="vfull")
            for b in range(8):
                r, sl = (b, 0) if b < 4 else (7 - b, 1)
                dma(out=v_full[:, b, :, :],
                    in_=kv_out[l][r*4 + 2 + sl].rearrange("p (h f) -> p h f", f=64))
            # ---- attention ----
            oT = [act.tile([128, 256], F32R, tag=f"oT{f}", name=f"oT{f}", bufs=1) for f in range(8)]
            for qb in range(2):
                if qb == 0:
                    chunks = [(0, 512, mAg if is_global else mAl)]
                elif is_global:
                    chunks = [(0, 512, None), (512, 512, mBg)]
                else:
                    chunks = [(256, 512, mBl[:, 0:512]), (768, 256, mBl[:, 512:768])]
                kn = sum(w for _, w, _ in chunks)
                ks = chunks[0][0]
                ops = [ppo.tile([128, 512], F32, tag=f"ops{t}", name=f"ops{t}", bufs=1) for t in range(2)]
                for h in range(HQ):
                    kvh = h // 4
                    base = (h % 2) * 64
                    ktile = kT_full[kvh // 2] if (kvh % 2) == (h % 2) else kT_sw[kvh // 2]
                    lhq = qT[h // 2][base:base+64, qb*128:(qb+1)*128]
                    probs = atp.tile([128, 1024], F16, tag="probs", name="probs")
                    accs = []
                    off = 0
                    for (c0, w, msk) in chunks:
                        sps = ppb.tile([128, 512], F32, tag="pb", name="sps")
                        nc.tensor.matmul(sps[:, 0:w], lhsT=lhq,
                                         rhs=ktile[base:base+64, c0:c0+w],
                                         start=True, stop=True)
                        if msk is not None:
                            nc.vector.tensor_add(out=sps[:, 0:w], in0=sps[:, 0:w], in1=msk)
                        acc = sm.tile([128, 1], F32, tag="acc", name="acc")
                        nc.scalar.activation(out=probs[:, off:off+w], in_=sps[:, 0:w],
                                             func=AF.Exp, scale=SCALE, accum_out=acc)
                        accs.append(acc)
                        off += w
                    if len(accs) == 2:
                        nc.vector.tensor_add(out=accs[0], in0=accs[0], in1=accs[1])
                    rec = sm.tile([128, 1], F32, tag="rec", name="rec")
                    nc.vector.reciprocal(out=rec, in_=accs[0])
                    nc.vector.tensor_scalar_mul(out=probs[:, 0:kn], in0=probs[:, 0:kn], scalar1=rec)
                    for jb in range(kn // 128):
                        b = ks // 128 + jb
                        pst = ppt.tile([128, 128], F16, tag="pstr", name="pstr")
                        nc.tensor.transpose(pst, probs[:, jb*128:(jb+1)*128], idn)
                        att = atp.tile([128, 128], F16, tag="attnT", name="attnT", bufs=3)
                        if jb % 2 == 0:
                            nc.vector.tensor_copy(out=att, in_=pst)
                        else:
                            nc.scalar.activation(out=att, in_=pst, func=AF.Copy)
                        g = (h // 2) % 4
                        nc.tensor.matmul(ops[h // 8][base:base+64, g*128:(g+1)*128],
                                         lhsT=v_full[:, b, kvh, :], rhs=att,
                                         start=(jb == 0), stop=(jb == kn // 128 - 1),
                                         tile_position=(0, base))
                for t_ in range(2):
                    for g in range(4):
                        if g % 2 == 0:
                            nc.vector.tensor_copy(out=oT[t_*4 + g][:, qb*128:(qb+1)*128],
                                                  in_=ops[t_][:, g*128:(g+1)*128])
                        else:
                            nc.scalar.activation(out=oT[t_*4 + g][:, qb*128:(qb+1)*128],
                                                 in_=ops[t_][:, g*128:(g+1)*128], func=AF.Copy)
            # ---- O proj ----
            for d in range(8):
                wot = wp.tile([128, 8, 128], F32R, tag="wot", name="wot")
                dma(out=wot, in_=P["wo"][l, :, d*128:(d+1)*128].rearrange("(ft p) c -> p ft c", p=128).bitcast(F32R))
                pso = ppm.tile([128, 256], F32, tag="pm", name="pso")
                for ft in range(8):
                    nc.tensor.matmul(pso, lhsT=wot[:, ft, :], rhs=oT[ft],
                                     start=(ft == 0), stop=(ft == 7))
                nc.vector.tensor_add(out=xT[d], in0=xT[d], in1=pso)
            # ---- FFN ----
            h2 = rmsnorm(P["n2h"][l])
            yT = []
            for (h0, hw) in HCHUNKS:
                w1t = wp.tile([128, 8, 128], F32R, tag="w1t", name="w1t")
                w3t = wp.tile([128, 8, 128], F32R, tag="w3t", name="w3t")
                dma(out=w1t[:, :, 0:hw], in_=P["w1"][l, :, h0:h0+hw].rearrange("(dc p) h -> p dc h", p=128).bitcast(F32R))
                dma(out=w3t[:, :, 0:hw], in_=P["w3"][l, :, h0:h0+hw].rearrange("(dc p) h -> p dc h", p=128).bitcast(F32R))
                psu = ppm.tile([128, 256], F32, tag="pm", name="psu")
                psg = ppm.tile([128, 256], F32, tag="pm", name="psg")
                for d in range(8):
                    nc.tensor.matmul(psu[0:hw, :], lhsT=w1t[:, d, 0:hw], rhs=h2[d],
                                     start=(d == 0), stop=(d == 7))
                for d in range(8):
                    nc.tensor.matmul(psg[0:hw, :], lhsT=w3t[:, d, 0:hw], rhs=h2[d],
                                     start=(d == 0), stop=(d == 7))
                su = act.tile([128, 256], F32, tag="su", name="su")
                nc.scalar.activation(out=su[0:hw, :], in_=psu[0:hw, :], func=AF.Silu)
                y = act.tile([128, 256], F16, tag=f"yT{h0}", name=f"yT{h0}", bufs=1)
                nc.vector.tensor_mul(out=y[0:hw, :], in0=su[0:hw, :], in1=psg[0:hw, :])
                yT.append(y)
            for d in range(8):
                w2t = wp.tile([128, 21, 128], F16, tag="w2t", name="w2t")
                dma(out=w2t, in_=P["w2"][l, 0:2688, d*128:(d+1)*128].rearrange("(hc p) c -> p hc c", p=128))
                w2x = wp.tile([64, 128], F16, tag="w2x", name="w2x")
                dma(out=w2x, in_=P["w2"][l, 2688:2752, d*128:(d+1)*128])
                ps2 = ppm.tile([128, 256], F32, tag="pm", name="ps2")
                nhc = len(HCHUNKS)
                for ci, (h0, hw) in enumerate(HCHUNKS):
                    lh = w2t[:, h0 // 128, :] if hw == 128 else w2x
                    nc.tensor.matmul(ps2, lhsT=lh, rhs=yT[ci][0:hw, :],
                                     start=(ci == 0), stop=(ci == nhc - 1))
                nc.vector.tensor_add(out=xT[d], in0=xT[d], in1=ps2)
        # ---- final ----
        xf = rmsnorm(P["nfh"][:, :], outdt=F16)
        for d in range(8):
            dma(out=xf_in[d*128:(d+1)*128, :], in_=xf[d])
        nc.gpsimd.collective_compute(
            "AllGather", mybir.AluOpType.bypass, replica_groups=RG_ALL,
            ins=[xf_in.ap()], outs=[xf_out.ap()])
        for vc in range(8):
            embt = wp.tile([128, 8, 500], F16, tag="embt", name="embt")
            dma(out=embt, in_=P["embT"][:, vc*500:(vc+1)*500].rearrange("(dc p) v -> p dc v", p=128))
            for tcn in range(16):
                r, sl = divmod(tcn, 2)
                xft = wp.tile([128, 8, 128], F16, tag="xft", name="xft")
                dma(out=xft, in_=xf_out[r*D:(r+1)*D, sl*128:(sl+1)*128].rearrange("(dc p) t -> p dc t", p=128))
                psl = ppb.tile([128, 500], F32, tag="pb", name="psl")
                for d in range(8):
                    nc.tensor.matmul(psl, lhsT=xft[:, d, :], rhs=embt[:, d, :],
                                     start=(d == 0), stop=(d == 7))
                lg = act.tile([128, 500], F32, tag="lg", name="lg")
                if tcn % 2 == 0:
                    nc.vector.tensor_copy(out=lg, in_=psl)
                else:
                    nc.scalar.activation(out=lg, in_=psl, func=AF.Copy)
                dma(out=logits[tcn*128:(tcn+1)*128, vc*500:(vc+1)*500], in_=lg)
    nc.compile()
    return nc

_NC_CACHE = {}
def _get_nc():
    if "nc" not in _NC_CACHE:
        _NC_CACHE["nc"] = _build_nc()
    return _NC_CACHE["nc"]

def kernel(**inputs):
    from concourse.bass_utils import run_bass_kernel_spmd
    nc = _get_nc()
    in_maps = _host_prep(**inputs)
    res = run_bass_kernel_spmd(nc, in_maps, list(range(NCORES)))
    return _assemble(res.results)

